# revision 25
# baseline (speedup 1.0000x reference)
"""AnyVariateAttention Trainium2 kernel (8 NeuronCores, SPMD, no collectives).

Problem: B=4, S=2048, D=512, H=8 attention with RoPE and a per-head
same-variate bias (u_same where variate_ids match, u_cross elsewhere),
softmax over keys, output projection.

Sharding: core c = 2*b + hf handles batch b and query-half hf (1024
queries, all 8 heads).  Attention rows are independent over queries, so
every core produces a disjoint slice of the output -- no collective.
To keep the SPMD graph identical across cores, x / ids / mask / key-side
rope tables are ROLLED host-side by the core's query offset (softmax and
PV are invariant to a permutation of the key axis), so the core's own
queries always sit in columns [0, 1024).

Algebraic folds (all exact w.r.t. the reference):
- variate bias: same[i,j] = onehot(id_i) . onehot(id_j) since ids in [0,8);
  scores+bias = [q*scale, oh_i, 1] . [k, du_h*oh_j, maskbias_j] with
  du_h = u_same[h]-u_cross[h]; the u_cross[h] term is uniform over j and
  cancels in softmax.  K extends 64 -> 73, free on the PE.
- mask: (1-mask_j)*-1e9 is the 73rd column (maskbias_j).
- softmax denominator: a ones column appended to V makes row 64 of the
  PV accumulation equal sum_j exp(s_ij); no standalone reduction needed.
- RoPE: rot(q) = q*C + swap_pairs(q)*S; swap_pairs folds into a second
  projection with pair-swapped weight rows.  The 1/sqrt(64) scale is
  pre-multiplied into the q-side tables.
- projection biases: K=1 matmuls (bias row x ones row) accumulated into
  the projection PSUM groups.

Everything is computed transposed (feature dim on partitions) so PE
emits scores^T directly and the softmax reduction folds into PV.
"""

import sys
import types

import numpy as np

# ---------------------------------------------------------------------------
# Environment patches (kernel.py must be self-contained).
# ---------------------------------------------------------------------------


def _install_patches():
    if 'antenv.axon_hooks' not in sys.modules:
        try:
            sys.path.insert(0, '/root/.axon_site/trn_agent_boot')
            import trn_boot
            hook = trn_boot._ntff_profile_via_ctypes('/opt/axon/libaxon_pjrt.so')
            mod = types.ModuleType('antenv.axon_hooks')
            mod.get_axon_ntff_profile_hook = lambda: hook
            mod.set_axon_ntff_profile_hook = lambda h: None
            sys.modules['antenv.axon_hooks'] = mod
        except Exception:
            pass

    # Walrus in this image accepts only one sync-wait on a CTRL (Drain)
    # instruction; TileContext's exit drain can carry several.  Spill the
    # extras onto following sync-engine nops (still before the all-engine
    # barrier, so semantics are unchanged).
    import concourse.tile as tile
    import concourse.mybir as mybir
    from concourse.vector_clock import ScopedClock

    if getattr(tile.TileContext, '_drain_patched', False):
        return

    def _drain_and_barrier(self, tick_clock, wait_clock):
        nc = self.nc
        drain_inst = nc.sync.drain()
        wait_clock.add_sem_waits(
            drain_inst.ins, ScopedClock({None: tick_clock.global_clock})
        )
        si = drain_inst.ins.sync_info
        waits = list(si.on_wait)
        if len(waits) > 1:
            si.on_wait = waits[:1]
            for w in waits[1:]:
                nop = nc.sync.nop()
                nop.ins.sync_info = mybir.SyncInfo(on_wait=[w], on_update=[])

        nc.all_engine_barrier()
        assert self.sems is not None
        popped = nc._tile_sem_poison_stack.pop()
        assert popped is self._sem_poison
        nc.clear_and_free_semaphores(list(self.sems.allocated().values()))
        nc.all_engine_barrier()

    tile.TileContext._drain_and_barrier = _drain_and_barrier
    tile.TileContext._drain_patched = True


_install_patches()

import concourse.bass as bass
import concourse.mybir as mybir
import concourse.tile as tile
from concourse.bass_utils import run_bass_kernel_spmd

# ---------------------------------------------------------------------------
# Problem constants (hardcoded per the spec).
# ---------------------------------------------------------------------------
B, S, D = 4, 2048, 512
H, HD = 8, 64
SI = S // 2      # queries per core
KE = HD + 8 + 1  # 73: extended contraction dim for scores
N_CORES = 8
ROPE_BASE = 10000.0
F32 = mybir.dt.float32
BF16 = mybir.dt.bfloat16

NKT = D // 128       # 4 k-tiles over model dim
NET = D // 128       # 4 e-tiles over projection dims (all 8 heads)
NST = S // 128       # 16 key tiles
VW = HD + 1          # 65: v plus ones column


def _rope_tables():
    inv_freq = 1.0 / (ROPE_BASE ** (np.arange(0, HD, 2, dtype=np.float64) / HD))
    t = np.arange(S, dtype=np.float64)
    freqs = np.outer(t, inv_freq)                  # (S, 32)
    emb = np.concatenate([freqs, freqs], axis=-1)  # (S, 64)
    ch = np.cos(emb)[:, ::2]                       # (S, 32)
    sh = np.sin(emb)[:, ::2]                       # (S, 32)
    C = np.empty((S, HD), dtype=np.float64)
    Sg = np.empty((S, HD), dtype=np.float64)
    C[:, 0::2] = ch
    C[:, 1::2] = ch
    Sg[:, 0::2] = -sh
    Sg[:, 1::2] = sh
    # transposed (64, S), tiled over the two heads of an e-tile -> (128, S)
    CT = np.tile(C.T, (2, 1))
    ST = np.tile(Sg.T, (2, 1))
    return CT, ST


def _bcast_ap(src, nparts):
    return bass.AP(tensor=src.tensor, offset=src.offset,
                   ap=[[0, nparts]] + [list(p) for p in src.ap][1:])


def build_graph():
    nc = bass.Bass(num_devices=N_CORES)

    xT = nc.declare_dram_parameter("xT", [D, S], F32, isOutput=False)
    WqT = nc.declare_dram_parameter("WqT", [D, D], F32, isOutput=False)
    WkT = nc.declare_dram_parameter("WkT", [D, D], F32, isOutput=False)
    WvT = nc.declare_dram_parameter("WvT", [D, D], F32, isOutput=False)
    WoT = nc.declare_dram_parameter("WoT", [D, D], F32, isOutput=False)
    pbias = nc.declare_dram_parameter("pbias", [8, D], F32, isOutput=False)
    CqTd = nc.declare_dram_parameter("CqT", [128, SI], F32, isOutput=False)
    SqTd = nc.declare_dram_parameter("SqT", [128, SI], F32, isOutput=False)
    CkTd = nc.declare_dram_parameter("CkT", [128, S], F32, isOutput=False)
    SkTd = nc.declare_dram_parameter("SkT", [128, S], F32, isOutput=False)
    ids = nc.declare_dram_parameter("ids", [1, S], F32, isOutput=False)
    maskin = nc.declare_dram_parameter("maskin", [1, S], F32, isOutput=False)
    iota8 = nc.declare_dram_parameter("iota8", [64, 1], F32, isOutput=False)
    du = nc.declare_dram_parameter("du", [64, 1], F32, isOutput=False)
    boh = nc.declare_dram_parameter("boh", [D, 1], F32, isOutput=False)
    permd = nc.declare_dram_parameter("perm128", [128, 128], F32,
                                      isOutput=False)
    pbcold = nc.declare_dram_parameter("pbcol", [128, 8], F32,
                                       isOutput=False)
    out_ext = nc.declare_dram_parameter("out", [D, SI], F32, isOutput=True)

    rec_dram = nc.dram_tensor("rec_dram", [1, SI], F32)

    with tile.TileContext(nc) as tc:
        with tc.tile_pool(name="persist", bufs=1) as pp:
            # persistent tensors
            xT_bf = pp.tile([128, NKT, S], BF16, tag="xT_bf")
            wq_bf = pp.tile([128, NKT, D], BF16, tag="wq")
            wk_bf = pp.tile([128, NKT, D], BF16, tag="wk")
            wv_bf = pp.tile([128, NKT, D], BF16, tag="wv")
            wo_bf = pp.tile([128, NET, D], BF16, tag="wo_bf")
            cq = pp.tile([128, SI], BF16, tag="cq")
            sq = pp.tile([128, SI], BF16, tag="sq")
            ck = pp.tile([128, S], BF16, tag="ck")
            sk = pp.tile([128, S], BF16, tag="sk")
            ones_bf = pp.tile([1, 512], BF16, tag="ones_bf")
            ones_row = pp.tile([1, SI], BF16, tag="ones_row")
            iota_sb = pp.tile([64, 1], F32, tag="iota_sb")
            du_sb = pp.tile([64, 1], F32, tag="du_sb")
            mb_bf = pp.tile([1, S], BF16, tag="mb_bf")
            bo_sb = pp.tile([128, NKT], F32, tag="bo_sb")
            perm_sb = pp.tile([128, 128], F32, tag="perm_sb")
            pb_col = pp.tile([128, 8], F32, tag="pb_col")
            qe = [pp.tile([KE, SI], BF16, tag=f"qe{h}", name=f"qe{h}")
                  for h in range(H)]
            ke = [pp.tile([KE, S], BF16, tag=f"ke{h}", name=f"ke{h}")
                  for h in range(H)]
            v_bf = pp.tile([128, NST, H, VW], BF16, tag="v_bf")
            o_bf = pp.tile([128, NET, 2, 512], BF16, tag="o_bf")
            pbb = [pp.tile([1, D], BF16, tag=f"pbb{r}", name=f"pbb{r}")
                   for r in range(5)]

            nc.vector.memset(ones_bf, 1.0)
            nc.vector.memset(ones_row, 1.0)
            nc.sync.dma_start(out=iota_sb, in_=iota8[:])
            nc.sync.dma_start(out=du_sb, in_=du[:])
            nc.sync.dma_start(out=perm_sb, in_=permd[:])
            nc.sync.dma_start(out=pb_col, in_=pbcold[:])
            for ft in range(NKT):
                nc.sync.dma_start(out=bo_sb[:, ft:ft + 1],
                                  in_=boh[ft * 128:(ft + 1) * 128, :])
            nc.vector.memset(v_bf[:, :, :, HD:VW], 1.0)

            with (
                tc.tile_pool(name="early", bufs=1) as ep,
                tc.tile_pool(name="psA", bufs=2, space="PSUM") as psA,
            ):
                # ---------------- phase A: loads + casts --------------------
                for kt in range(NKT):
                    xst = ep.tile([128, S], F32, tag="xstage", bufs=2)
                    nc.sync.dma_start(out=xst,
                                      in_=xT[kt * 128:(kt + 1) * 128, :])
                    nc.scalar.activation(xT_bf[:, kt, :], xst[:, :],
                                         mybir.ActivationFunctionType.Copy)

                for ext, wb in ((WqT, wq_bf), (WkT, wk_bf),
                                (WvT, wv_bf), (WoT, wo_bf)):
                    for kt in range(NKT):
                        wst = ep.tile([128, D], F32, tag="wstage", bufs=2)
                        nc.sync.dma_start(out=wst,
                                          in_=ext[kt * 128:(kt + 1) * 128, :])
                        nc.scalar.activation(wb[:, kt, :], wst[:, :],
                                             mybir.ActivationFunctionType.Copy)

                for ext, dst in ((CkTd, ck), (SkTd, sk)):
                    tst = ep.tile([128, S], F32, tag="xstage", bufs=2)
                    nc.sync.dma_start(out=tst, in_=ext[:])
                    nc.scalar.activation(dst[:, :], tst[:, :],
                                         mybir.ActivationFunctionType.Copy)
                for ext, dst in ((CqTd, cq), (SqTd, sq)):
                    tstq = ep.tile([128, SI], F32, tag="tstageq", bufs=2)
                    nc.sync.dma_start(out=tstq, in_=ext[:])
                    nc.scalar.activation(dst[:, :], tstq[:, :],
                                         mybir.ActivationFunctionType.Copy)

                for r in range(5):
                    pbf_r = ep.tile([1, D], F32, tag="pbstage", bufs=1,
                                    name=f"pbf{r}")
                    nc.sync.dma_start(out=pbf_r, in_=pbias[r:r + 1, :])
                    nc.vector.tensor_copy(pbb[r][:, :], pbf_r[:, :])

                ids_bc = ep.tile([64, S], F32, tag="ids_bc", bufs=1)
                nc.sync.dma_start(out=ids_bc, in_=_bcast_ap(ids[:], 64))
                oh_bf = ep.tile([8, S], BF16, tag="oh_bf")
                nc.vector.tensor_scalar(oh_bf, ids_bc[0:8, :],
                                        iota_sb[0:8, :], None,
                                        op0=mybir.AluOpType.is_equal)
                ohdu64 = ep.tile([64, S], BF16, tag="ohdu64", bufs=1)
                nc.vector.tensor_scalar(ohdu64, ids_bc, iota_sb[:],
                                        du_sb[:, :],
                                        op0=mybir.AluOpType.is_equal,
                                        op1=mybir.AluOpType.mult)
                mask_sb = ep.tile([1, S], F32, tag="xstage", bufs=2)
                nc.sync.dma_start(out=mask_sb, in_=maskin[:])
                # Copy(1e9*mask - 1e9) = -1e9*(1-mask)
                nc.scalar.activation(mb_bf, mask_sb,
                                     mybir.ActivationFunctionType.Copy,
                                     bias=-1e9, scale=1e9)

                for h in range(H):
                    nc.sync.dma_start(out=qe[h][HD:HD + 8, :],
                                      in_=oh_bf[:, 0:SI])
                    nc.sync.dma_start(out=qe[h][HD + 8:KE, :],
                                      in_=ones_row[:, :])
                    nc.sync.dma_start(out=ke[h][HD:HD + 8, :],
                                      in_=ohdu64[8 * h:8 * h + 8, :])
                    nc.sync.dma_start(out=ke[h][HD + 8:KE, :],
                                      in_=mb_bf[:, :])

                # ---------------- phase B: projections + rope ---------------
                for st in range(NST):
                    pv = psA.tile([128, 512], F32, tag="pv", bufs=4,
                                  name=f"pv{st}")
                    for kt in range(NKT):
                        nc.tensor.matmul(
                            pv[:, 0:512],
                            xT_bf[:, kt, st * 128:(st + 1) * 128],
                            wv_bf[:, kt, :],
                            start=kt == 0, stop=False)
                    nc.tensor.matmul(
                        pv[:, 0:512],
                        ones_bf[:, 0:128],
                        pbb[4][:, :],
                        start=False, stop=True)
                    nc.scalar.activation(
                        v_bf[:, st, :, 0:HD],
                        pv[:, 0:512].rearrange("p (h d) -> p h d", h=H),
                        mybir.ActivationFunctionType.Copy)

                # q-side: out (D_e, SI_s); k-side: out (D_e, S_s)
                # rot(q)[d] = q[d]*C[d] + q[d^1]*S[d]; the S-tables arrive
                # row-pair-swapped, so t2[d] = q[d]*S[d^1] and the stride-2
                # adds below read t2 at d^1.
                for et in range(NET):
                    e0 = et * 128
                    for which in range(2):  # 0: q, 1: k
                        w0 = (wq_bf, wk_bf)[which]
                        bcol = which * 4
                        ctab = (cq, ck)[which]
                        stab = (sq, sk)[which]
                        dst = (qe, ke)[which]
                        slen = (SI, S)[which]
                        for sc in range(slen // 1024):
                            s0 = sc * 1024
                            p0 = psA.tile([128, 1024], F32, tag="pproj",
                                          name=f"p0_{et}_{which}_{sc}")
                            for half in range(2):
                                hs = s0 + half * 512
                                o0 = half * 512
                                for kt in range(NKT):
                                    nc.tensor.matmul(
                                        p0[:, o0:o0 + 512],
                                        w0[:, kt, e0:e0 + 128],
                                        xT_bf[:, kt, hs:hs + 512],
                                        start=kt == 0, stop=kt == NKT - 1)
                            t1 = ep.tile([128, 1024], F32, tag="ropet1",
                                         bufs=2, name=f"t1_{et}_{which}_{sc}")
                            t2 = ep.tile([128, 1024], F32, tag="ropet2",
                                         bufs=2, name=f"t2_{et}_{which}_{sc}")
                            stg = ep.tile([128, 1024], BF16, tag="ropstg",
                                          bufs=2, name=f"sg_{et}_{which}_{sc}")
                            nc.vector.scalar_tensor_tensor(
                                t2, p0[:, :],
                                pb_col[:, bcol + et:bcol + et + 1],
                                stab[:, s0:s0 + 1024],
                                op0=mybir.AluOpType.add,
                                op1=mybir.AluOpType.mult)
                            psw = psA.tile([128, 1024], F32, tag="pproj",
                                           name=f"pw_{et}_{which}_{sc}")
                            for half in range(2):
                                nc.tensor.matmul(
                                    psw[:, half * 512:half * 512 + 512],
                                    perm_sb[:, :],
                                    t2[:, half * 512:half * 512 + 512],
                                    start=True, stop=True)
                            nc.vector.scalar_tensor_tensor(
                                t1, p0[:, :],
                                pb_col[:, bcol + et:bcol + et + 1],
                                ctab[:, s0:s0 + 1024],
                                op0=mybir.AluOpType.add,
                                op1=mybir.AluOpType.mult)
                            nc.vector.tensor_add(stg, t1, psw[:, :])
                            nc.sync.dma_start(
                                out=dst[et * 2][0:HD, s0:s0 + 1024],
                                in_=stg[0:64, :])
                            nc.sync.dma_start(
                                out=dst[et * 2 + 1][0:HD, s0:s0 + 1024],
                                in_=stg[64:128, :])

            # ---------------- phase C: attention per head -------------------
            with (
                tc.tile_pool(name="late", bufs=1) as lp,
                tc.tile_pool(name="psS", bufs=2, space="PSUM") as psS,
                tc.tile_pool(name="psO", bufs=2, space="PSUM") as psO,
            ):
                for h in range(H):
                    et, hh = h // 2, h % 2
                    po = psO.tile([VW, 2, 512], F32, tag="po",
                                  name=f"po{h}")
                    for jt in range(NST):
                        ps = psS.tile([128, 2, 512], F32, tag="ps",
                                      name=f"ps{h}_{jt}")
                        for q2 in range(2):
                            nc.tensor.matmul(
                                ps[:, q2, :],
                                ke[h][:, jt * 128:(jt + 1) * 128],
                                qe[h][:, q2 * 512:(q2 + 1) * 512],
                                start=True, stop=True)
                        probs = lp.tile([128, 2, 512], BF16, tag="probs",
                                        bufs=3, name=f"pr{h}_{jt}")
                        nc.scalar.activation(probs, ps[:, :, :],
                                             mybir.ActivationFunctionType.Exp)
                        for q2 in range(2):
                            nc.tensor.matmul(po[:, q2, :],
                                             v_bf[:, jt, h, :],
                                             probs[:, q2, :],
                                             start=jt == 0,
                                             stop=jt == NST - 1)
                    # normalize: o = numer / denom (denom = row 64 of po)
                    den_row = lp.tile([1, 2, 512], F32, tag="den_row",
                                      bufs=2, name=f"dr{h}")
                    nc.vector.tensor_copy(den_row, po[HD:VW, :, :])
                    numer = lp.tile([64, 2, 512], BF16, tag="numer",
                                    bufs=2, name=f"nm{h}")
                    nc.vector.tensor_copy(numer, po[0:HD, :, :])
                    rec_mat = lp.tile([128, 8], F32, tag="rec_mat", bufs=2,
                                      name=f"rm{h}")
                    nc.sync.dma_start(out=rec_mat, in_=den_row[:, :, :])
                    rinv = lp.tile([128, 8], F32, tag="rinv", bufs=2,
                                   name=f"ri{h}")
                    nc.vector.reciprocal(rinv, rec_mat[:, :])
                    nc.sync.dma_start(out=rec_dram[:], in_=rinv[:, :])
                    rec_bc = lp.tile([64, 2, 512], F32, tag="rec_bc", bufs=2,
                                     name=f"rb{h}")
                    nc.sync.dma_start(out=rec_bc,
                                      in_=_bcast_ap(rec_dram[:], 64))
                    nc.vector.tensor_mul(o_bf[hh * 64:hh * 64 + 64, et, :, :],
                                         numer[:, :, :], rec_bc[:, :, :])

            # ---------------- phase D: output projection --------------------
            with (
                tc.tile_pool(name="fin", bufs=1) as fp,
                tc.tile_pool(name="psD", bufs=2, space="PSUM") as psD,
            ):
                for ft in range(NKT):
                    for sc in range(2):
                        s0 = sc * 512
                        pout = psD.tile([128, 512], F32, tag="pout",
                                        name=f"pd_{ft}_{sc}")
                        for et in range(NET):
                            nc.tensor.matmul(
                                pout,
                                wo_bf[:, et, ft * 128:(ft + 1) * 128],
                                o_bf[:, et, sc, :],
                                start=et == 0, stop=et == NET - 1)
                        outf = fp.tile([128, 512], F32, tag="outf",
                                       bufs=2, name=f"of{ft}_{sc}")
                        nc.vector.tensor_scalar_add(outf, pout,
                                                    bo_sb[:, ft:ft + 1])
                        nc.sync.dma_start(
                            out=out_ext[ft * 128:(ft + 1) * 128,
                                        s0:s0 + 512],
                            in_=outf)

    _spill_sync_waits(nc)
    return nc


def _spill_sync_waits(nc, max_waits=1):
    """Walrus in this image allows very few sync-wait commands per
    instruction.  Hoist extras onto same-engine nops placed just before the
    instruction (same blocking semantics on the engine's sequencer)."""
    for bb in nc.cur_f.blocks:
        new = []
        changed = False
        for inst in bb.instructions:
            si = inst.sync_info
            waits = list(si.on_wait) if si is not None else []
            if len(waits) > max_waits:
                for w in waits[:-max_waits]:
                    nop = mybir.InstNoOp(name=f"spillw-{nc.next_id()}",
                                         engine=inst.engine, ins=[], outs=[])
                    nop.sync_info = mybir.SyncInfo(on_wait=[w], on_update=[])
                    new.append(nop)
                si.on_wait = waits[-max_waits:]
                changed = True
            new.append(inst)
        if changed:
            bb.instructions = new


_GRAPH = None


def _get_graph():
    global _GRAPH
    if _GRAPH is None:
        _GRAPH = build_graph()
    return _GRAPH


def make_in_maps(x, variate_ids, mask, Wq, bq, Wk, bk, Wv, bv, Wo, bo,
                 u_same, u_cross):
    CT, ST = _rope_tables()
    scale = 1.0 / np.sqrt(HD)
    perm = np.arange(D) ^ 1  # pair swap
    iota = np.arange(8, dtype=np.float32).reshape(8, 1)
    duv = (np.asarray(u_same) - np.asarray(u_cross)).astype(np.float32)

    x = np.asarray(x, dtype=np.float32)
    variate_ids = np.asarray(variate_ids)
    mask = np.asarray(mask, dtype=np.float32)
    Wq, Wk, Wv, Wo = (np.asarray(a, dtype=np.float32) for a in (Wq, Wk, Wv, Wo))
    bq, bk, bv, bo = (np.asarray(a, dtype=np.float32) for a in (bq, bk, bv, bo))

    pb = np.zeros((8, D), dtype=np.float32)
    pb[0] = bq
    pb[2] = bk
    pb[4] = bv
    iota64 = np.tile(np.arange(8, dtype=np.float32), H).reshape(64, 1)
    du64 = np.repeat(duv, 8).astype(np.float32).reshape(64, 1)
    rperm = np.arange(128) ^ 1  # row pair swap for the S tables
    WqTh = np.ascontiguousarray(Wq.T)
    WkTh = np.ascontiguousarray(Wk.T)
    WvTh = np.ascontiguousarray(Wv.T)
    WoTh = np.ascontiguousarray(Wo.T)

    in_maps = []
    for c in range(N_CORES):
        b, hf = c // 2, c % 2
        off = hf * SI

        def roll(a):
            return np.roll(a, -off, axis=-1)

        in_maps.append({
            "xT": np.ascontiguousarray(roll(x[b].T)),
            "WqT": WqTh, "WkT": WkTh, "WvT": WvTh, "WoT": WoTh,
            "pbias": pb,
            "CqT": np.ascontiguousarray(
                (CT[:, off:off + SI] * scale).astype(np.float32)),
            "SqT": np.ascontiguousarray(
                (ST[rperm][:, off:off + SI] * scale).astype(np.float32)),
            "CkT": np.ascontiguousarray(roll(CT).astype(np.float32)),
            "SkT": np.ascontiguousarray(roll(ST[rperm]).astype(np.float32)),
            "ids": roll(variate_ids[b].astype(np.float32)).reshape(1, S),
            "maskin": roll(mask[b]).reshape(1, S),
            "iota8": iota64,
            "du": du64,
            "boh": bo.reshape(D, 1),
            "perm128": np.eye(128, dtype=np.float32)[np.arange(128) ^ 1],
            "pbcol": np.concatenate([bq.reshape(4, 128).T,
                                     bk.reshape(4, 128).T], axis=1),
        })
    return in_maps


def run(inputs, trace=False):
    nc = _get_graph()
    in_maps = make_in_maps(**inputs)
    res = run_bass_kernel_spmd(nc, in_maps, list(range(N_CORES)), trace=trace)
    out = np.empty((B, S, D), dtype=np.float32)
    for c in range(N_CORES):
        b, hf = c // 2, c % 2
        out[b, hf * SI:(hf + 1) * SI, :] = res.results[c]["out"].T
    return out, res


def kernel(**inputs) -> np.ndarray:
    out, _ = run(inputs, trace=False)
    return out


# revision 26
# speedup vs baseline: 1.1998x; 1.1998x over previous
"""AnyVariateAttention Trainium2 kernel (8 NeuronCores, SPMD, no collectives).

Problem: B=4, S=2048, D=512, H=8 attention with RoPE and a per-head
same-variate bias (u_same where variate_ids match, u_cross elsewhere),
softmax over keys, output projection.

Sharding: core c = 2*b + hf handles batch b and query-half hf (1024
queries, all 8 heads).  Attention rows are independent over queries, so
every core produces a disjoint slice of the output -- no collective.
To keep the SPMD graph identical across cores, x / ids / mask / key-side
rope tables are ROLLED host-side by the core's query offset (softmax and
PV are invariant to a permutation of the key axis), so the core's own
queries always sit in columns [0, 1024).

Algebraic folds (all exact w.r.t. the reference):
- variate bias: same[i,j] = onehot(id_i) . onehot(id_j) since ids in [0,8);
  scores+bias = [q*scale, oh_i, 1] . [k, du_h*oh_j, maskbias_j] with
  du_h = u_same[h]-u_cross[h]; the u_cross[h] term is uniform over j and
  cancels in softmax.  K extends 64 -> 73, free on the PE.
- mask: (1-mask_j)*-1e9 is the 73rd column (maskbias_j).
- softmax denominator: a ones column appended to V makes row 64 of the
  PV accumulation equal sum_j exp(s_ij); no standalone reduction needed.
- RoPE: rot(q) = q*C + swap_pairs(q)*S; swap_pairs folds into a second
  projection with pair-swapped weight rows.  The 1/sqrt(64) scale is
  pre-multiplied into the q-side tables.
- projection biases: K=1 matmuls (bias row x ones row) accumulated into
  the projection PSUM groups.

Everything is computed transposed (feature dim on partitions) so PE
emits scores^T directly and the softmax reduction folds into PV.
"""

import sys
import types

import numpy as np

# ---------------------------------------------------------------------------
# Environment patches (kernel.py must be self-contained).
# ---------------------------------------------------------------------------


def _install_patches():
    if 'antenv.axon_hooks' not in sys.modules:
        try:
            sys.path.insert(0, '/root/.axon_site/trn_agent_boot')
            import trn_boot
            hook = trn_boot._ntff_profile_via_ctypes('/opt/axon/libaxon_pjrt.so')
            mod = types.ModuleType('antenv.axon_hooks')
            mod.get_axon_ntff_profile_hook = lambda: hook
            mod.set_axon_ntff_profile_hook = lambda h: None
            sys.modules['antenv.axon_hooks'] = mod
        except Exception:
            pass

    # Walrus in this image accepts only one sync-wait on a CTRL (Drain)
    # instruction; TileContext's exit drain can carry several.  Spill the
    # extras onto following sync-engine nops (still before the all-engine
    # barrier, so semantics are unchanged).
    import concourse.tile as tile
    import concourse.mybir as mybir
    from concourse.vector_clock import ScopedClock

    if getattr(tile.TileContext, '_drain_patched', False):
        return

    def _drain_and_barrier(self, tick_clock, wait_clock):
        nc = self.nc
        drain_inst = nc.sync.drain()
        wait_clock.add_sem_waits(
            drain_inst.ins, ScopedClock({None: tick_clock.global_clock})
        )
        si = drain_inst.ins.sync_info
        waits = list(si.on_wait)
        if len(waits) > 1:
            si.on_wait = waits[:1]
            for w in waits[1:]:
                nop = nc.sync.nop()
                nop.ins.sync_info = mybir.SyncInfo(on_wait=[w], on_update=[])

        nc.all_engine_barrier()
        assert self.sems is not None
        popped = nc._tile_sem_poison_stack.pop()
        assert popped is self._sem_poison
        nc.clear_and_free_semaphores(list(self.sems.allocated().values()))
        nc.all_engine_barrier()

    tile.TileContext._drain_and_barrier = _drain_and_barrier
    tile.TileContext._drain_patched = True


_install_patches()

import concourse.bass as bass
import concourse.mybir as mybir
import concourse.tile as tile
from concourse.bass_utils import run_bass_kernel_spmd

# ---------------------------------------------------------------------------
# Problem constants (hardcoded per the spec).
# ---------------------------------------------------------------------------
B, S, D = 4, 2048, 512
H, HD = 8, 64
SI = S // 2      # queries per core
KE = HD + 8 + 1  # 73: extended contraction dim for scores
N_CORES = 8
ROPE_BASE = 10000.0
F32 = mybir.dt.float32
BF16 = mybir.dt.bfloat16

NKT = D // 128       # 4 k-tiles over model dim
NET = D // 128       # 4 e-tiles over projection dims (all 8 heads)
NST = S // 128       # 16 key tiles
VW = HD + 1          # 65: v plus ones column


def _rope_tables():
    inv_freq = 1.0 / (ROPE_BASE ** (np.arange(0, HD, 2, dtype=np.float64) / HD))
    t = np.arange(S, dtype=np.float64)
    freqs = np.outer(t, inv_freq)                  # (S, 32)
    emb = np.concatenate([freqs, freqs], axis=-1)  # (S, 64)
    ch = np.cos(emb)[:, ::2]                       # (S, 32)
    sh = np.sin(emb)[:, ::2]                       # (S, 32)
    C = np.empty((S, HD), dtype=np.float64)
    Sg = np.empty((S, HD), dtype=np.float64)
    C[:, 0::2] = ch
    C[:, 1::2] = ch
    Sg[:, 0::2] = -sh
    Sg[:, 1::2] = sh
    # transposed (64, S), tiled over the two heads of an e-tile -> (128, S)
    CT = np.tile(C.T, (2, 1))
    ST = np.tile(Sg.T, (2, 1))
    return CT, ST


def _bcast_ap(src, nparts):
    return bass.AP(tensor=src.tensor, offset=src.offset,
                   ap=[[0, nparts]] + [list(p) for p in src.ap][1:])


def build_graph():
    nc = bass.Bass(num_devices=N_CORES)

    xT = nc.declare_dram_parameter("xT", [D, S], F32, isOutput=False)
    WqT = nc.declare_dram_parameter("WqT", [D, D], F32, isOutput=False)
    WkT = nc.declare_dram_parameter("WkT", [D, D], F32, isOutput=False)
    WvT = nc.declare_dram_parameter("WvT", [D, D], F32, isOutput=False)
    WoT = nc.declare_dram_parameter("WoT", [D, D], F32, isOutput=False)
    pbias = nc.declare_dram_parameter("pbias", [8, D], F32, isOutput=False)
    CqTd = nc.declare_dram_parameter("CqT", [128, SI], F32, isOutput=False)
    SqTd = nc.declare_dram_parameter("SqT", [128, SI], F32, isOutput=False)
    CkTd = nc.declare_dram_parameter("CkT", [128, S], F32, isOutput=False)
    SkTd = nc.declare_dram_parameter("SkT", [128, S], F32, isOutput=False)
    ids = nc.declare_dram_parameter("ids", [1, S], F32, isOutput=False)
    maskin = nc.declare_dram_parameter("maskin", [1, S], F32, isOutput=False)
    iota8 = nc.declare_dram_parameter("iota8", [64, 1], F32, isOutput=False)
    du = nc.declare_dram_parameter("du", [64, 1], F32, isOutput=False)
    boh = nc.declare_dram_parameter("boh", [D, 1], F32, isOutput=False)
    permd = nc.declare_dram_parameter("perm128", [128, 128], F32,
                                      isOutput=False)
    pbcold = nc.declare_dram_parameter("pbcol", [128, 8], F32,
                                       isOutput=False)
    out_ext = nc.declare_dram_parameter("out", [D, SI], F32, isOutput=True)

    rec_dram = nc.dram_tensor("rec_dram", [1, SI], F32)

    with tile.TileContext(nc) as tc:
        with tc.tile_pool(name="persist", bufs=1) as pp:
            # persistent tensors
            xT_bf = pp.tile([128, NKT, S], BF16, tag="xT_bf")
            wq_bf = pp.tile([128, NKT, D], BF16, tag="wq")
            wk_bf = pp.tile([128, NKT, D], BF16, tag="wk")
            wv_bf = pp.tile([128, NKT, D], BF16, tag="wv")
            wo_bf = pp.tile([128, NET, D], BF16, tag="wo_bf")
            cq = pp.tile([128, SI], BF16, tag="cq")
            sq = pp.tile([128, SI], BF16, tag="sq")
            ck = pp.tile([128, S], BF16, tag="ck")
            sk = pp.tile([128, S], BF16, tag="sk")
            ones_bf = pp.tile([1, 512], BF16, tag="ones_bf")
            ones_row = pp.tile([1, SI], BF16, tag="ones_row")
            iota_sb = pp.tile([64, 1], F32, tag="iota_sb")
            du_sb = pp.tile([64, 1], F32, tag="du_sb")
            mb_bf = pp.tile([1, S], BF16, tag="mb_bf")
            bo_sb = pp.tile([128, NKT], F32, tag="bo_sb")
            perm_sb = pp.tile([128, 128], F32, tag="perm_sb")
            pb_col = pp.tile([128, 8], F32, tag="pb_col")
            qe = [pp.tile([KE, SI], BF16, tag=f"qe{h}", name=f"qe{h}")
                  for h in range(H)]
            ke = [pp.tile([KE, S], BF16, tag=f"ke{h}", name=f"ke{h}")
                  for h in range(H)]
            v_bf = pp.tile([128, NST, H, VW], BF16, tag="v_bf")
            o_bf = pp.tile([128, NET, 2, 512], BF16, tag="o_bf")
            pbb = [pp.tile([1, D], BF16, tag=f"pbb{r}", name=f"pbb{r}")
                   for r in range(5)]

            nc.vector.memset(ones_bf, 1.0)
            nc.vector.memset(ones_row, 1.0)
            nc.sync.dma_start(out=iota_sb, in_=iota8[:])
            nc.sync.dma_start(out=du_sb, in_=du[:])
            nc.sync.dma_start(out=perm_sb, in_=permd[:])
            nc.sync.dma_start(out=pb_col, in_=pbcold[:])
            for ft in range(NKT):
                nc.sync.dma_start(out=bo_sb[:, ft:ft + 1],
                                  in_=boh[ft * 128:(ft + 1) * 128, :])
            nc.vector.memset(v_bf[:, :, :, HD:VW], 1.0)

            with (
                tc.tile_pool(name="early", bufs=1) as ep,
                tc.tile_pool(name="psA", bufs=2, space="PSUM") as psA,
            ):
                # ---------------- phase A: loads + casts --------------------
                for kt in range(NKT):
                    xst = ep.tile([128, S], F32, tag="xstage", bufs=2)
                    nc.sync.dma_start(out=xst,
                                      in_=xT[kt * 128:(kt + 1) * 128, :])
                    nc.scalar.activation(xT_bf[:, kt, :], xst[:, :],
                                         mybir.ActivationFunctionType.Copy)

                for ext, wb in ((WqT, wq_bf), (WkT, wk_bf),
                                (WvT, wv_bf), (WoT, wo_bf)):
                    for kt in range(NKT):
                        wst = ep.tile([128, D], F32, tag="wstage", bufs=2)
                        nc.sync.dma_start(out=wst,
                                          in_=ext[kt * 128:(kt + 1) * 128, :])
                        nc.scalar.activation(wb[:, kt, :], wst[:, :],
                                             mybir.ActivationFunctionType.Copy)

                for ext, dst in ((CkTd, ck), (SkTd, sk)):
                    tst = ep.tile([128, S], F32, tag="xstage", bufs=2)
                    nc.sync.dma_start(out=tst, in_=ext[:])
                    nc.scalar.activation(dst[:, :], tst[:, :],
                                         mybir.ActivationFunctionType.Copy)
                for ext, dst in ((CqTd, cq), (SqTd, sq)):
                    tstq = ep.tile([128, SI], F32, tag="tstageq", bufs=2)
                    nc.sync.dma_start(out=tstq, in_=ext[:])
                    nc.scalar.activation(dst[:, :], tstq[:, :],
                                         mybir.ActivationFunctionType.Copy)

                for r in range(5):
                    pbf_r = ep.tile([1, D], F32, tag="pbstage", bufs=1,
                                    name=f"pbf{r}")
                    nc.sync.dma_start(out=pbf_r, in_=pbias[r:r + 1, :])
                    nc.vector.tensor_copy(pbb[r][:, :], pbf_r[:, :])

                ids_bc = ep.tile([64, S], F32, tag="ids_bc", bufs=1)
                nc.sync.dma_start(out=ids_bc, in_=_bcast_ap(ids[:], 64))
                oh_bf = ep.tile([8, S], BF16, tag="oh_bf")
                nc.vector.tensor_scalar(oh_bf, ids_bc[0:8, :],
                                        iota_sb[0:8, :], None,
                                        op0=mybir.AluOpType.is_equal)
                ohdu64 = ep.tile([64, S], BF16, tag="ohdu64", bufs=1)
                nc.vector.tensor_scalar(ohdu64, ids_bc, iota_sb[:],
                                        du_sb[:, :],
                                        op0=mybir.AluOpType.is_equal,
                                        op1=mybir.AluOpType.mult)
                mask_sb = ep.tile([1, S], F32, tag="xstage", bufs=2)
                nc.sync.dma_start(out=mask_sb, in_=maskin[:])
                # Copy(1e9*mask - 1e9) = -1e9*(1-mask)
                nc.scalar.activation(mb_bf, mask_sb,
                                     mybir.ActivationFunctionType.Copy,
                                     bias=-1e9, scale=1e9)

                for h in range(H):
                    nc.sync.dma_start(out=qe[h][HD:HD + 8, :],
                                      in_=oh_bf[:, 0:SI])
                    nc.sync.dma_start(out=qe[h][HD + 8:KE, :],
                                      in_=ones_row[:, :])
                    nc.sync.dma_start(out=ke[h][HD:HD + 8, :],
                                      in_=ohdu64[8 * h:8 * h + 8, :])
                    nc.sync.dma_start(out=ke[h][HD + 8:KE, :],
                                      in_=mb_bf[:, :])

                # ---------------- phase B: projections + rope ---------------
                for st in range(NST):
                    pv = psA.tile([128, 512], F32, tag="pv", bufs=4,
                                  name=f"pv{st}")
                    for kt in range(NKT):
                        nc.tensor.matmul(
                            pv[:, 0:512],
                            xT_bf[:, kt, st * 128:(st + 1) * 128],
                            wv_bf[:, kt, :],
                            start=kt == 0, stop=False)
                    nc.tensor.matmul(
                        pv[:, 0:512],
                        ones_bf[:, 0:128],
                        pbb[4][:, :],
                        start=False, stop=True)
                    nc.scalar.activation(
                        v_bf[:, st, :, 0:HD],
                        pv[:, 0:512].rearrange("p (h d) -> p h d", h=H),
                        mybir.ActivationFunctionType.Copy)

                # q-side: out (D_e, SI_s); k-side: out (D_e, S_s)
                # rot(q)[d] = q[d]*C[d] + q[d^1]*S[d]; the S-tables arrive
                # row-pair-swapped, so t2[d] = q[d]*S[d^1] and the stride-2
                # adds below read t2 at d^1.
                for et in range(NET):
                    e0 = et * 128
                    for which in range(2):  # 0: q, 1: k
                        w0 = (wq_bf, wk_bf)[which]
                        bcol = which * 4
                        ctab = (cq, ck)[which]
                        stab = (sq, sk)[which]
                        dst = (qe, ke)[which]
                        slen = (SI, S)[which]
                        for sc in range(slen // 1024):
                            s0 = sc * 1024
                            p0 = psA.tile([128, 1024], F32, tag="pproj",
                                          name=f"p0_{et}_{which}_{sc}")
                            for half in range(2):
                                hs = s0 + half * 512
                                o0 = half * 512
                                for kt in range(NKT):
                                    nc.tensor.matmul(
                                        p0[:, o0:o0 + 512],
                                        w0[:, kt, e0:e0 + 128],
                                        xT_bf[:, kt, hs:hs + 512],
                                        start=kt == 0, stop=kt == NKT - 1)
                            t1 = ep.tile([128, 1024], F32, tag="ropet1",
                                         bufs=2, name=f"t1_{et}_{which}_{sc}")
                            t2 = ep.tile([128, 1024], F32, tag="ropet2",
                                         bufs=2, name=f"t2_{et}_{which}_{sc}")
                            stg = ep.tile([128, 1024], BF16, tag="ropstg",
                                          bufs=2, name=f"sg_{et}_{which}_{sc}")
                            nc.vector.scalar_tensor_tensor(
                                t2, p0[:, :],
                                pb_col[:, bcol + et:bcol + et + 1],
                                stab[:, s0:s0 + 1024],
                                op0=mybir.AluOpType.add,
                                op1=mybir.AluOpType.mult)
                            t2s = ep.tile([128, 1024], F32, tag="ropet2s",
                                          bufs=2, name=f"t2s_{et}_{which}_{sc}")
                            nc.sync.dma_start(out=t2s[0:128:2, :],
                                              in_=t2[1:128:2, :])
                            nc.sync.dma_start(out=t2s[1:128:2, :],
                                              in_=t2[0:128:2, :])
                            nc.vector.scalar_tensor_tensor(
                                t1, p0[:, :],
                                pb_col[:, bcol + et:bcol + et + 1],
                                ctab[:, s0:s0 + 1024],
                                op0=mybir.AluOpType.add,
                                op1=mybir.AluOpType.mult)
                            nc.vector.tensor_add(stg, t1, t2s[:, :])
                            nc.sync.dma_start(
                                out=dst[et * 2][0:HD, s0:s0 + 1024],
                                in_=stg[0:64, :])
                            nc.sync.dma_start(
                                out=dst[et * 2 + 1][0:HD, s0:s0 + 1024],
                                in_=stg[64:128, :])

            # ---------------- phase C: attention per head -------------------
            with (
                tc.tile_pool(name="late", bufs=1) as lp,
                tc.tile_pool(name="psS", bufs=2, space="PSUM") as psS,
                tc.tile_pool(name="psO", bufs=2, space="PSUM") as psO,
            ):
                for h in range(H):
                    et, hh = h // 2, h % 2
                    po = psO.tile([VW, 2, 512], F32, tag="po",
                                  name=f"po{h}")
                    for jt in range(NST):
                        ps = psS.tile([128, 2, 512], F32, tag="ps",
                                      name=f"ps{h}_{jt}")
                        for q2 in range(2):
                            nc.tensor.matmul(
                                ps[:, q2, :],
                                ke[h][:, jt * 128:(jt + 1) * 128],
                                qe[h][:, q2 * 512:(q2 + 1) * 512],
                                start=True, stop=True)
                        probs = lp.tile([128, 2, 512], BF16, tag="probs",
                                        bufs=3, name=f"pr{h}_{jt}")
                        nc.scalar.activation(probs, ps[:, :, :],
                                             mybir.ActivationFunctionType.Exp)
                        for q2 in range(2):
                            nc.tensor.matmul(po[:, q2, :],
                                             v_bf[:, jt, h, :],
                                             probs[:, q2, :],
                                             start=jt == 0,
                                             stop=jt == NST - 1)
                    # normalize: o = numer / denom (denom = row 64 of po)
                    den_row = lp.tile([1, 2, 512], F32, tag="den_row",
                                      bufs=2, name=f"dr{h}")
                    nc.vector.tensor_copy(den_row, po[HD:VW, :, :])
                    numer = lp.tile([64, 2, 512], BF16, tag="numer",
                                    bufs=2, name=f"nm{h}")
                    nc.vector.tensor_copy(numer, po[0:HD, :, :])
                    rec_mat = lp.tile([128, 8], F32, tag="rec_mat", bufs=2,
                                      name=f"rm{h}")
                    nc.sync.dma_start(out=rec_mat, in_=den_row[:, :, :])
                    rinv = lp.tile([128, 8], F32, tag="rinv", bufs=2,
                                   name=f"ri{h}")
                    nc.vector.reciprocal(rinv, rec_mat[:, :])
                    nc.sync.dma_start(out=rec_dram[:], in_=rinv[:, :])
                    rec_bc = lp.tile([64, 2, 512], F32, tag="rec_bc", bufs=2,
                                     name=f"rb{h}")
                    nc.sync.dma_start(out=rec_bc,
                                      in_=_bcast_ap(rec_dram[:], 64))
                    nc.vector.tensor_mul(o_bf[hh * 64:hh * 64 + 64, et, :, :],
                                         numer[:, :, :], rec_bc[:, :, :])

            # ---------------- phase D: output projection --------------------
            with (
                tc.tile_pool(name="fin", bufs=1) as fp,
                tc.tile_pool(name="psD", bufs=2, space="PSUM") as psD,
            ):
                for ft in range(NKT):
                    for sc in range(2):
                        s0 = sc * 512
                        pout = psD.tile([128, 512], F32, tag="pout",
                                        name=f"pd_{ft}_{sc}")
                        for et in range(NET):
                            nc.tensor.matmul(
                                pout,
                                wo_bf[:, et, ft * 128:(ft + 1) * 128],
                                o_bf[:, et, sc, :],
                                start=et == 0, stop=et == NET - 1)
                        outf = fp.tile([128, 512], F32, tag="outf",
                                       bufs=2, name=f"of{ft}_{sc}")
                        nc.vector.tensor_scalar_add(outf, pout,
                                                    bo_sb[:, ft:ft + 1])
                        nc.sync.dma_start(
                            out=out_ext[ft * 128:(ft + 1) * 128,
                                        s0:s0 + 512],
                            in_=outf)

    _spill_sync_waits(nc)
    return nc


def _spill_sync_waits(nc, max_waits=1):
    """Walrus in this image allows very few sync-wait commands per
    instruction.  Hoist extras onto same-engine nops placed just before the
    instruction (same blocking semantics on the engine's sequencer)."""
    for bb in nc.cur_f.blocks:
        new = []
        changed = False
        for inst in bb.instructions:
            si = inst.sync_info
            waits = list(si.on_wait) if si is not None else []
            if len(waits) > max_waits:
                for w in waits[:-max_waits]:
                    nop = mybir.InstNoOp(name=f"spillw-{nc.next_id()}",
                                         engine=inst.engine, ins=[], outs=[])
                    nop.sync_info = mybir.SyncInfo(on_wait=[w], on_update=[])
                    new.append(nop)
                si.on_wait = waits[-max_waits:]
                changed = True
            new.append(inst)
        if changed:
            bb.instructions = new


_GRAPH = None


def _get_graph():
    global _GRAPH
    if _GRAPH is None:
        _GRAPH = build_graph()
    return _GRAPH


def make_in_maps(x, variate_ids, mask, Wq, bq, Wk, bk, Wv, bv, Wo, bo,
                 u_same, u_cross):
    CT, ST = _rope_tables()
    scale = 1.0 / np.sqrt(HD)
    perm = np.arange(D) ^ 1  # pair swap
    iota = np.arange(8, dtype=np.float32).reshape(8, 1)
    duv = (np.asarray(u_same) - np.asarray(u_cross)).astype(np.float32)

    x = np.asarray(x, dtype=np.float32)
    variate_ids = np.asarray(variate_ids)
    mask = np.asarray(mask, dtype=np.float32)
    Wq, Wk, Wv, Wo = (np.asarray(a, dtype=np.float32) for a in (Wq, Wk, Wv, Wo))
    bq, bk, bv, bo = (np.asarray(a, dtype=np.float32) for a in (bq, bk, bv, bo))

    pb = np.zeros((8, D), dtype=np.float32)
    pb[0] = bq
    pb[2] = bk
    pb[4] = bv
    iota64 = np.tile(np.arange(8, dtype=np.float32), H).reshape(64, 1)
    du64 = np.repeat(duv, 8).astype(np.float32).reshape(64, 1)
    rperm = np.arange(128) ^ 1  # row pair swap for the S tables
    WqTh = np.ascontiguousarray(Wq.T)
    WkTh = np.ascontiguousarray(Wk.T)
    WvTh = np.ascontiguousarray(Wv.T)
    WoTh = np.ascontiguousarray(Wo.T)

    in_maps = []
    for c in range(N_CORES):
        b, hf = c // 2, c % 2
        off = hf * SI

        def roll(a):
            return np.roll(a, -off, axis=-1)

        in_maps.append({
            "xT": np.ascontiguousarray(roll(x[b].T)),
            "WqT": WqTh, "WkT": WkTh, "WvT": WvTh, "WoT": WoTh,
            "pbias": pb,
            "CqT": np.ascontiguousarray(
                (CT[:, off:off + SI] * scale).astype(np.float32)),
            "SqT": np.ascontiguousarray(
                (ST[rperm][:, off:off + SI] * scale).astype(np.float32)),
            "CkT": np.ascontiguousarray(roll(CT).astype(np.float32)),
            "SkT": np.ascontiguousarray(roll(ST[rperm]).astype(np.float32)),
            "ids": roll(variate_ids[b].astype(np.float32)).reshape(1, S),
            "maskin": roll(mask[b]).reshape(1, S),
            "iota8": iota64,
            "du": du64,
            "boh": bo.reshape(D, 1),
            "perm128": np.eye(128, dtype=np.float32)[np.arange(128) ^ 1],
            "pbcol": np.concatenate([bq.reshape(4, 128).T,
                                     bk.reshape(4, 128).T], axis=1),
        })
    return in_maps


def run(inputs, trace=False):
    nc = _get_graph()
    in_maps = make_in_maps(**inputs)
    res = run_bass_kernel_spmd(nc, in_maps, list(range(N_CORES)), trace=trace)
    out = np.empty((B, S, D), dtype=np.float32)
    for c in range(N_CORES):
        b, hf = c // 2, c % 2
        out[b, hf * SI:(hf + 1) * SI, :] = res.results[c]["out"].T
    return out, res


def kernel(**inputs) -> np.ndarray:
    out, _ = run(inputs, trace=False)
    return out
